# revision 18
# baseline (speedup 1.0000x reference)
# Multi-head causal self-attention (B=2, S=2048, H=16, D=64) on 8 TRN2 cores.
#
# Sharding: batch*head parallel. Core c handles batch b=c//4 and head group
# g=c%4 (heads 4g..4g+4, i.e. 256 of the 1024 hidden channels).
#   - Q/K/V projections column-parallel over heads (each core computes its
#     256 output channels from the full hidden states of its batch).
#   - Scores/softmax/PV local per head shard.
#   - Output projection row-parallel: each core computes a full [S, HID]
#     partial product from its 256 attn channels; host sums the 4 partials
#     per batch (and adds bo).
#
# Device kernel (per core), all matmuls bf16 with fp32 PSUM accumulation:
#   1. x loaded NATURALLY (contiguous DMA, 4 x 1MB chunks), then transposed
#      on-chip with PE transpose matmuls (128x128 tiles via identity),
#      PSUM -> SBUF copies split between ScalarE and VectorE. No DMA
#      transposes (DRAM-side xbar transpose degenerates to tiny strided
#      descriptors on this stack: ~170 MB/s measured).
#   2. qT,kT = W{q,k}T-slice @ xT  (+bias);  v = xT.T @ WvT-slice (+bias),
#      stored with an appended ones column per head ("v_aug").
#   3. Per head, per 512-wide query block: scoresT[t,sq] tiles via PE
#      (K=64 contraction; head pairs use partitions 0-63/64-127), exp on
#      ScalarE (scale=1/8 fused; no max subtraction -- logits are O(1)),
#      causal masking on diagonal tiles via a multiplicative 0/1 bf16 mask
#      on VectorE (no gpsimd), PV accumulates attn_augT[65, sq] where
#      row 64 = softmax denominator.
#   4. Normalize: r = 1/denominator (DVE), broadcast across 64 partitions via
#      a K=1 PE matmul with a ones vector, multiply.
#   5. out_partial[s,:] = attnT.T @ WoT-slice -> bf16 -> DRAM.

import numpy as np

S = 2048
HID = 1024
D = 64
HPC = 4  # heads per core
M = HPC * D  # 256 local channels
DT = HID // 128  # 8 d-tiles
ST = S // 128  # 16 s-tiles
QB = 512  # query block width
NQB = S // QB  # 4 query blocks
SCALE = 0.125  # 1/sqrt(64)

_CACHE = {}


def _build_bass(
    n_repeat=1,
    phases=("x", "proj", "attn", "oproj"),
    skew=2,
    mask_gpsimd=True,
    split_attn=False,
    pt_bufs=4,
    copies_dve=False,
    interleave=False,
):
    import concourse.bass as bass
    import concourse.mybir as mybir
    import concourse.tile as tile
    from concourse import bacc

    FP = mybir.dt.float32
    BF = mybir.dt.bfloat16
    Exp = mybir.ActivationFunctionType.Exp
    mult = mybir.AluOpType.mult
    add = mybir.AluOpType.add

    nc = bacc.Bacc("TRN2", target_bir_lowering=False)

    x_d = nc.dram_tensor("x", [S, HID], BF, kind="ExternalInput")
    wq_d = nc.dram_tensor("wq_t", [HID, M], BF, kind="ExternalInput")
    wk_d = nc.dram_tensor("wk_t", [HID, M], BF, kind="ExternalInput")
    wv_d = nc.dram_tensor("wv_t", [HID, M], BF, kind="ExternalInput")
    wo_d = nc.dram_tensor("wo_t", [M, HID], BF, kind="ExternalInput")
    bq_d = nc.dram_tensor("bq", [M], FP, kind="ExternalInput")
    bk_d = nc.dram_tensor("bk", [M], FP, kind="ExternalInput")
    bv_d = nc.dram_tensor("bv_rep", [128, M], FP, kind="ExternalInput")
    mask_d = nc.dram_tensor("mask01", [128, 128], BF, kind="ExternalInput")
    id_d = nc.dram_tensor("ident", [128, 128], BF, kind="ExternalInput")
    out_d = nc.dram_tensor("out_p", [S, HID], BF, kind="ExternalOutput")

    with tile.TileContext(nc) as tc:
        with (
            tc.tile_pool(name="const", bufs=1) as cpool,
            tc.tile_pool(name="pt", bufs=pt_bufs) as pt_pool,
            tc.tile_pool(name="rn", bufs=2) as rn_pool,
            tc.tile_pool(name="ob", bufs=3) as ob_pool,
            tc.tile_pool(name="ps_proj", bufs=2, space="PSUM") as ps_proj,
            tc.tile_pool(name="ps_sc", bufs=2, space="PSUM") as ps_sc,
            tc.tile_pool(name="ps_at", bufs=2, space="PSUM") as ps_at,
        ):
            # ---- persistent SBUF tensors ----
            wq_sb = cpool.tile([128, DT, M], BF, tag="wq")
            wk_sb = cpool.tile([128, DT, M], BF, tag="wk")
            wv_sb = cpool.tile([128, DT, M], BF, tag="wv")
            wo_sb = cpool.tile([128, 2, HID], BF, tag="wo")
            bq_sb = cpool.tile([128, 2], FP, tag="bq")
            bk_sb = cpool.tile([128, 2], FP, tag="bk")
            bvr_sb = cpool.tile([128, M], FP, tag="bvr")
            ones_sb = cpool.tile([1, 64], FP, tag="ones")
            mask_sb = cpool.tile([128, 128], BF, tag="mask")
            id_sb = cpool.tile([128, 128], BF, tag="ident")
            xn_sb = cpool.tile([128, ST, HID], BF, tag="xn")
            xt_sb = cpool.tile([128, DT, S], BF, tag="xt")
            qt_sb = cpool.tile([128, 2, S], BF, tag="qt")
            kt_sb = cpool.tile([128, 2, S], BF, tag="kt")
            vaug_sb = cpool.tile([128, ST, HPC, D + 1], BF, tag="vaug")
            att_sb = cpool.tile([128, 2, S], BF, tag="att")

            # ---- constants / weights (outside the repeat loop) ----
            nc.sync.dma_start(wq_sb[:], wq_d.rearrange("(t p) m -> p t m", p=128))
            nc.sync.dma_start(wk_sb[:], wk_d.rearrange("(t p) m -> p t m", p=128))
            nc.sync.dma_start(wv_sb[:], wv_d.rearrange("(t p) m -> p t m", p=128))
            nc.sync.dma_start(wo_sb[:], wo_d.rearrange("(t p) e -> p t e", p=128))
            nc.sync.dma_start(bq_sb[:], bq_d.rearrange("(t p) -> p t", p=128))
            nc.sync.dma_start(bk_sb[:], bk_d.rearrange("(t p) -> p t", p=128))
            nc.sync.dma_start(bvr_sb[:], bv_d[:])
            nc.sync.dma_start(mask_sb[:], mask_d[:])
            nc.sync.dma_start(id_sb[:], id_d[:])
            nc.vector.memset(ones_sb[:], 1.0)
            nc.vector.memset(vaug_sb[:, :, :, D], 1.0)

            from contextlib import nullcontext

            ET = mybir.EngineType
            loop = (
                tc.For_i(
                    0,
                    n_repeat,
                    1,
                    staggered_reset=True,
                    hint_engines=(ET.PE, ET.Activation, ET.DVE),
                )
                if n_repeat > 1
                else nullcontext()
            )
            with loop:
                rep = 0

                def _stage():
                    if n_repeat > 1:
                        tc.stage_boundary()
                # ---- load x naturally; PE-transpose to xT [d, s] ----
                if "x" in phases:
                    for sg in range(4):
                        eng = nc.sync if sg % 2 == 0 else nc.scalar
                        eng.dma_start(
                            xn_sb[:, 4 * sg : 4 * (sg + 1), :],
                            x_d[512 * sg : 512 * (sg + 1), :].rearrange(
                                "(t p) d -> p t d", p=128
                            ),
                        )
                    ci = 0
                    for sg in range(4):
                        for dg in range(DT // 2):
                            # two d-tiles x four s-tiles of 128x128 transposes
                            # packed into one bf16 PSUM bank -> one big copy
                            tp = ps_proj.tile(
                                [128, 2, 4, 128], BF, tag="proj",
                                name=f"tp{sg}_{dg}_{rep}",
                            )
                            for k in range(8):
                                dt, i = 2 * dg + k // 4, k % 4
                                nc.tensor.transpose(
                                    tp[:, k // 4, i, :],
                                    xn_sb[:, 4 * sg + i, 128 * dt : 128 * (dt + 1)],
                                    id_sb[:],
                                )
                            dst = xt_sb[:, 2 * dg : 2 * dg + 2, 512 * sg : 512 * (sg + 1)]
                            src = tp[:].rearrange("p a b c -> p a (b c)")
                            if copies_dve or ci % 2 == 0:
                                nc.vector.tensor_copy(dst, src)
                            else:
                                nc.scalar.copy(dst, src)
                            ci += 1

                # ---- q/k/v projection building blocks ----
                def _qkproj(sc):
                    for w_sb, b_sb, o_sb in (
                        (wq_sb, bq_sb, qt_sb),
                        (wk_sb, bk_sb, kt_sb),
                    ):
                        for mt in range(2):
                            ps = ps_proj.tile([128, 512], FP, tag="proj")
                            for kt_i in range(DT):
                                nc.tensor.matmul(
                                    ps[:],
                                    w_sb[:, kt_i, 128 * mt : 128 * (mt + 1)],
                                    xt_sb[:, kt_i, 512 * sc : 512 * (sc + 1)],
                                    start=(kt_i == 0),
                                    stop=(kt_i == DT - 1),
                                )
                            nc.vector.tensor_scalar_add(
                                o_sb[:, mt, 512 * sc : 512 * (sc + 1)],
                                ps[:],
                                b_sb[:, mt : mt + 1],
                            )

                def _vproj(st):
                    ps = ps_proj.tile([128, M], FP, tag="proj", name=f"vps{st}_{rep}")
                    for kt_i in range(DT):
                        nc.tensor.matmul(
                            ps[:],
                            xt_sb[:, kt_i, 128 * st : 128 * (st + 1)],
                            wv_sb[:, kt_i, :],
                            start=(kt_i == 0),
                            stop=(kt_i == DT - 1),
                        )
                    nc.vector.tensor_tensor(
                        vaug_sb[:, st, :, 0:D],
                        ps[:].rearrange("p (h d) -> p h d", h=HPC),
                        bvr_sb[:].rearrange("p (h d) -> p h d", h=HPC),
                        add,
                    )

                if not interleave:
                    _stage()  # stage 1: q/k/v projections
                    if "proj" in phases:
                        for sc in range(4):
                            _qkproj(sc)
                        for st in range(ST):
                            _vproj(st)

                # ---- attention: head pairs share PE row groups ----
                SKEW = skew  # PV trails scores/exp by this many key tiles

                def _attn_group(qb, hp):
                    h0, h1 = 2 * hp, 2 * hp + 1
                    q0 = QB * qb
                    tmax = (q0 + QB) // 128
                    at_ps = {}
                    for h in (h0, h1):
                        at_ps[h] = ps_at.tile(
                            [D + 1, QB], FP, tag="at", name=f"at{h}_{qb}_{rep}"
                        )
                    pts = {}

                    def _pv(T):
                        c0 = max(0, 128 * T - q0)
                        for j, h in enumerate((h0, h1)):
                            nc.tensor.matmul(
                                at_ps[h][:, c0:],
                                vaug_sb[:, T, h, :],
                                pts[T][:, j, c0:],
                                start=(T == 0),
                                stop=(T == tmax - 1),
                            )
                        del pts[T]

                    for T in range(tmax):
                        c0 = max(0, 128 * T - q0)  # first valid col in block
                        sp = ps_sc.tile([128, 2, 512], FP, tag="sc")
                        for j, h in enumerate((h0, h1)):
                            lo = 64 * (h % 2)
                            nc.tensor.matmul(
                                sp[:, j, c0:QB],
                                kt_sb[lo : lo + 64, hp, 128 * T : 128 * (T + 1)],
                                qt_sb[lo : lo + 64, hp, q0 + c0 : q0 + QB],
                                start=True,
                                stop=True,
                            )
                        pt = pt_pool.tile([128, 2, QB], BF, tag="pt")
                        pts[T] = pt
                        nc.scalar.activation(
                            pt[:, :, c0:], sp[:, :, c0:], Exp, scale=SCALE
                        )
                        if 128 * T >= q0:  # diagonal tile: zero p where sq < t
                            meng = nc.gpsimd if mask_gpsimd else nc.vector
                            for j in (0, 1):
                                meng.tensor_tensor(
                                    pt[:, j, c0 : c0 + 128],
                                    pt[:, j, c0 : c0 + 128],
                                    mask_sb[:],
                                    mult,
                                )
                        if T >= SKEW:
                            _pv(T - SKEW)
                    for T in range(max(0, tmax - SKEW), tmax):
                        _pv(T)
                    # normalize: att = at_ps[0:64] / at_ps[64]
                    for h in (h0, h1):
                        lo = 64 * (h % 2)
                        r = rn_pool.tile([1, QB], FP, tag="r")
                        nc.vector.reciprocal(r[:], at_ps[h][D : D + 1, :])
                        rb_ps = ps_proj.tile(
                            [64, QB], FP, tag="proj", name=f"rb{h}_{qb}_{rep}"
                        )
                        nc.tensor.matmul(
                            rb_ps[:], ones_sb[:], r[:], start=True, stop=True
                        )
                        rb_sb = rn_pool.tile([64, QB], BF, tag="rbs")
                        nc.scalar.copy(rb_sb[:], rb_ps[:])
                        nc.vector.tensor_tensor(
                            att_sb[lo : lo + 64, hp, q0 : q0 + QB],
                            at_ps[h][0:D, :],
                            rb_sb[:],
                            mult,
                        )

                def _oproj_sg(sg):
                    ob = ob_pool.tile([128, 4, 2, 512], BF, tag="ob")
                    for si in range(4):
                        st = 4 * sg + si
                        for ec in range(2):
                            op = ps_proj.tile(
                                [128, 512], FP, tag="proj", name=f"op{st}_{ec}_{rep}"
                            )
                            for ct in range(2):
                                nc.tensor.matmul(
                                    op[:],
                                    att_sb[:, ct, 128 * st : 128 * (st + 1)],
                                    wo_sb[:, ct, 512 * ec : 512 * (ec + 1)],
                                    start=(ct == 0),
                                    stop=(ct == 1),
                                )
                            if copies_dve or (si + ec) % 2 == 0:
                                nc.vector.tensor_copy(ob[:, si, ec, :], op[:])
                            else:
                                nc.scalar.copy(ob[:, si, ec, :], op[:])
                    nc.sync.dma_start(
                        out_d[512 * sg : 512 * (sg + 1), :].rearrange(
                            "(q p) (a b) -> p q a b", p=128, a=2
                        ),
                        ob[:],
                    )

                if interleave:
                    # proj work interleaved with attention so PE-heavy
                    # projections overlap ACT-heavy softmax
                    P = "proj" in phases
                    A = "attn" in phases
                    O = "oproj" in phases
                    _stage()  # stage 1: sc0 projections + attn qb0
                    if P:
                        _qkproj(0)
                        for st in range(0, 4):
                            _vproj(st)
                    if A:
                        _attn_group(0, 0)
                        _attn_group(0, 1)
                    _stage()  # stage 2: sc1-3 projections + attn qb1-2
                    if P:
                        _qkproj(1)
                        for st in range(4, 8):
                            _vproj(st)
                    if A:
                        _attn_group(1, 0)
                    if P:
                        _qkproj(2)
                    if A:
                        _attn_group(1, 1)
                    if P:
                        _qkproj(3)
                        for st in range(8, 12):
                            _vproj(st)
                    if A:
                        _attn_group(2, 0)
                    if P:
                        for st in range(12, 16):
                            _vproj(st)
                    if A:
                        _attn_group(2, 1)
                    _stage()  # stage 3: attn qb3 + output projection
                    if A:
                        _attn_group(3, 0)
                    if O:
                        _oproj_sg(0)
                    if A:
                        _attn_group(3, 1)
                    if O:
                        _oproj_sg(1)
                        _oproj_sg(2)
                        _oproj_sg(3)
                elif split_attn:
                    _stage()  # stage 2: attention qb 0-1
                    if "attn" in phases:
                        for qb in (0, 1):
                            for hp in range(2):
                                _attn_group(qb, hp)
                    _stage()  # stage 3: attention qb 2-3 + oproj interleaved
                    if "attn" in phases:
                        _attn_group(2, 0)
                        _attn_group(2, 1)
                        if "oproj" in phases:
                            _oproj_sg(0)
                        _attn_group(3, 0)
                        if "oproj" in phases:
                            _oproj_sg(1)
                        _attn_group(3, 1)
                        if "oproj" in phases:
                            _oproj_sg(2)
                            _oproj_sg(3)
                    elif "oproj" in phases:
                        for sg in range(4):
                            _oproj_sg(sg)
                else:
                    _stage()  # stage 2: attention
                    if "attn" in phases:
                        for qb in range(NQB):
                            for hp in range(2):
                                _attn_group(qb, hp)
                    _stage()  # stage 3: output projection + store
                    if "oproj" in phases:
                        for sg in range(4):
                            _oproj_sg(sg)

    nc.compile()
    return nc


BUILD_OPTS = {}


def _get_bass(n_repeat=1, phases=("x", "proj", "attn", "oproj")):
    key = ("nc", n_repeat, tuple(phases), tuple(sorted(BUILD_OPTS.items())))
    if key not in _CACHE:
        _CACHE[key] = _build_bass(n_repeat, phases, **BUILD_OPTS)
    return _CACHE[key]


def _in_maps(inputs):
    import ml_dtypes

    bf = ml_dtypes.bfloat16
    hs = np.asarray(inputs["hidden_states"], dtype=np.float32).astype(bf)
    Wq = np.asarray(inputs["Wq"], dtype=np.float32).astype(bf)
    Wk = np.asarray(inputs["Wk"], dtype=np.float32).astype(bf)
    Wv = np.asarray(inputs["Wv"], dtype=np.float32).astype(bf)
    Wo = np.asarray(inputs["Wo"], dtype=np.float32).astype(bf)
    bq = np.asarray(inputs["bq"], dtype=np.float32)
    bk = np.asarray(inputs["bk"], dtype=np.float32)
    bv = np.asarray(inputs["bv"], dtype=np.float32)
    i = np.arange(128)
    mask01 = (i[:, None] <= i[None, :]).astype(bf)  # keep where q >= key
    ident = np.eye(128, dtype=np.float32).astype(bf)
    maps = []
    for c in range(8):
        b, g = c // 4, c % 4
        sl = slice(M * g, M * (g + 1))
        maps.append(
            {
                "x": np.ascontiguousarray(hs[b]),
                "wq_t": np.ascontiguousarray(Wq[sl, :].T),
                "wk_t": np.ascontiguousarray(Wk[sl, :].T),
                "wv_t": np.ascontiguousarray(Wv[sl, :].T),
                "wo_t": np.ascontiguousarray(Wo[:, sl].T),
                "bq": np.ascontiguousarray(bq[sl]),
                "bk": np.ascontiguousarray(bk[sl]),
                "bv_rep": np.ascontiguousarray(np.broadcast_to(bv[sl], (128, M))),
                "mask01": mask01,
                "ident": ident,
            }
        )
    return maps


def run(trace=False, n_repeat=1, **inputs):
    from concourse.bass_utils import run_bass_kernel_spmd

    nc = _get_bass(n_repeat)
    maps = _in_maps(inputs)
    res = run_bass_kernel_spmd(nc, maps, core_ids=list(range(8)), trace=trace)
    bo = np.asarray(inputs["bo"], dtype=np.float32)
    out = np.zeros((2, S, HID), np.float32)
    for c in range(8):
        out[c // 4] += res.results[c]["out_p"].astype(np.float32)
    out += bo[None, None, :]
    return out, res


def kernel(**inputs):
    out, _ = run(trace=False, **inputs)
    return out
